# revision 93
# baseline (speedup 1.0000x reference)
"""Trainium2 Bass kernel for DeepseekAttention (T=4096, H=2048, 16 heads, d=128).

Tensor-parallel over heads: 8 NeuronCores x 2 heads each (SPMD, per-core
inputs).  Host side: transpose hidden, split hidden/w_qkv/w_o into
error-compensated fp8 e4m3 hi+lo pairs (x ~= hi + lo, each quantized to
e4m3; weights pre-scaled x32 so they sit in e4m3's normal range), slice
per core, and precompute RoPE cos/sin tables (x 1/32) + a [128,128]
triangular mask block.

All projections run as fp8 DoubleRow matmuls (0.5 cycles/row vs 1.0 for
bf16): each k-tile pair needs 3 matmuls (Whi*Xhi + Wlo*Xhi + Whi*Xlo,
with the pair of k-tiles or the pair of heads stacked in the DoubleRow
slot dim), a net 25% PE saving at ~1e-3 relative error.  Attention
(S = K^T Q and P V) stays f16: a compensated fp8 S/PV costs as much PE
as f16, and single fp8 there breaks the 2e-2 accuracy gate.  The x32
V-weight scale is deliberately kept in VV so the normalized attention
(32*attn) lands in e4m3's sweet spot for the w_o hi/lo split; the final
1/1024 descale rides the eviction copies.

Per core, a software-pipelined loop over 8 T-chunks of 512:
  step c: PE runs the QKV projection of chunk c while ACT runs exp() of
  chunk c-1's attention scores (one instruction per k-tile PAIR via
  2-bank psum tiles, halving ACT access overhead), DVE accumulates
  softmax denominators, and GPSIMD reduces them across partitions.  The
  w_o projection of chunk c-2 is interleaved into the attention stream
  as PE filler; leftovers roll into the next step.  attn(7,0) runs
  inside step 7 so the final step only carries attn(7,1)+w_o instead of
  an exp-bound tail.
Cold-start: the timeline models ONE serial DMA pipe and a serial SWDGE
descriptor-gen engine, and the scheduler hoists dependency-free DMAs to
their queue front; dummy WAR reads gate the const/wo8 and hid(1) loads
so the chunk-0 stream and the tiny rope-rotate DMAs keep pipe priority.
Partial outputs (rows of A @ w_o) are summed across cores on the host.
"""

import numpy as np
import ml_dtypes
from collections import deque

import concourse.tile as tile
from concourse import bacc, mybir, bass_isa
from concourse.bass_utils import run_bass_kernel_spmd

T = 4096
HID = 2048
NHEADS = 16
HD = 128
NCORES = 8
HPC = NHEADS // NCORES        # 2 heads per core
FEAT = HPC * HD               # 256 per-core attention features
QKVF = 3 * FEAT               # 768 per-core qkv features
CH = 512                      # T-chunk width
NCH = T // CH                 # 8 chunks
KT = HID // 128               # 16 hidden k-tiles
SCALE = float(HD) ** -0.5
MASK_NEG = -30000.0
LAG = 5                       # S->exp->PV pipeline depth (in k-tiles)

BF = mybir.dt.bfloat16
F16 = mybir.dt.float16
F32 = mybir.dt.float32
F8 = mybir.dt.float8e4
NPF8 = ml_dtypes.float8_e4m3
DR = mybir.MatmulPerfMode.DoubleRow
WSCALE = 32.0                 # fp8 pre-scale on w_qkv/w_o (host); descale at evict


def _build_bass():
    nc = bacc.Bacc("TRN2", target_bir_lowering=False, debug=False,
                   num_devices=NCORES)

    # hid/wqkv are fp8 hi+lo pairs, interleaved at k-tile granularity:
    # rows (kt, two, p) so one DMA gulp covers hi+lo of nk k-tiles.
    hidT8 = nc.dram_tensor("hidT8", [2 * HID, T], F8, kind="ExternalInput").ap()
    wqkv8qk = nc.dram_tensor("wqkv8qk", [2 * HID, 512], F8,
                             kind="ExternalInput").ap()
    # v columns partition-major (SBUF image): 128 descs with >=4KB runs,
    # dodging the <512B-run 2x DMA latency penalty of 256-col slices
    wqkv8v = nc.dram_tensor("wqkv8v", [128, 2 * KT * 256], F8,
                            kind="ExternalInput").ap()
    # wo fp8 hi/lo, layout per partition (two, head, f), pre-scaled x32
    wo8 = nc.dram_tensor("wo8", [128, 4 * HID], F8, kind="ExternalInput").ap()
    cos2 = nc.dram_tensor("cos2", [128, T], F16, kind="ExternalInput").ap()
    sin2 = nc.dram_tensor("sin2", [128, T], F16, kind="ExternalInput").ap()
    mask = nc.dram_tensor("mask", [128, 128], F32, kind="ExternalInput").ap()
    out = nc.dram_tensor("out", [T, HID], F16, kind="ExternalOutput").ap()

    with tile.TileContext(nc) as tc:
        _emit(tc, hidT8, wqkv8qk, wqkv8v, wo8, cos2, sin2, mask, out)
    nc.compile()
    return nc


def _emit(tc, hidT8, wqkv8qk, wqkv8v, wo8, cos2, sin2, mask, out):
    nc = tc.nc
    from contextlib import ExitStack
    ctx = ExitStack()
    with ctx:
        const = ctx.enter_context(tc.tile_pool(name="const", bufs=1))
        hidp = ctx.enter_context(tc.tile_pool(name="hidp", bufs=2))
        ropep = ctx.enter_context(tc.tile_pool(name="ropep", bufs=2))
        ptp = ctx.enter_context(tc.tile_pool(name="ptp", bufs=5))
        accp = ctx.enter_context(tc.tile_pool(name="accp", bufs=2))
        qkp = ctx.enter_context(tc.tile_pool(name="qkp", bufs=2))
        atp = ctx.enter_context(tc.tile_pool(name="atp", bufs=2))
        persist = ctx.enter_context(tc.tile_pool(name="persist", bufs=1))
        stgp = ctx.enter_context(tc.tile_pool(name="stgp", bufs=3))
        # PSUM: 8 banks = S-ring 4 + po 1 + qk/v chain 1 + wo fillers 2
        psS = ctx.enter_context(tc.tile_pool(name="psS", bufs=4, space="PSUM"))
        psO = ctx.enter_context(tc.tile_pool(name="psO", bufs=1, space="PSUM"))
        psB = ctx.enter_context(tc.tile_pool(name="psB", bufs=1, space="PSUM"))

        # ---- constants ----
        ones_col = const.tile([128, 1], F16, tag="ones_col")
        nc.any.memset(ones_col[:], 1.0)
        # wqkv hi/lo fp8, k-tile-interleaved (k two f), in two regions
        # (q/k cols, v cols); compensated DoubleRow matmuls consume
        # [128, 2(k-tile), f] slices of the stride-2 3D views below.
        wq8qk_sb = const.tile([128, 2 * KT * 512], F8, tag="wq8qk_sb")
        wq8v_sb = const.tile([128, 2 * KT * 256], F8, tag="wq8v_sb")
        _wqk4 = wq8qk_sb[:].rearrange("p (k two f) -> p two k f", two=2, k=KT)
        WH3, WL3 = _wqk4[:, 0], _wqk4[:, 1]
        _wv4 = wq8v_sb[:].rearrange("p (k two f) -> p two k f", two=2, k=KT)
        WVH3, WVL3 = _wv4[:, 0], _wv4[:, 1]

        def load_wqkv_gulp(kt, nk=2):
            """nk k-tiles of wqkv hi+lo q/k columns on the Pool queue."""
            wv = wqkv8qk[kt * 256:(kt + nk) * 256, :].rearrange(
                "(k p) f -> p k f", p=128)
            nc.gpsimd.dma_start(
                wq8qk_sb[:, kt * 1024:(kt + nk) * 1024].rearrange(
                    "p (k f) -> p k f", k=2 * nk), wv)

        def load_wqkv_v(half):
            """Half of the partition-major v columns: one 128-desc DMA."""
            sl = slice(half * KT * 256, (half + 1) * KT * 256)
            nc.gpsimd.dma_start(wq8v_sb[:, sl], wqkv8v[:, sl])
        cos_sb = const.tile([128, T], F16, tag="cos_sb")
        sin_sb = const.tile([128, T], F16, tag="sin_sb")
        mask_sb = const.tile([128, 128], F32, tag="mask_sb")
        # wo fp8 hi/lo, (two, head, f) per partition; DoubleRow slots = heads
        wo8_sb = const.tile([128, 4 * HID], F8, tag="wo8_sb")
        _wo4 = wo8_sb[:].rearrange("p (two h f) -> p two h f", two=2, h=HPC)
        WOH, WOL = _wo4[:, 0], _wo4[:, 1]

        def load_consts():
            # chunk-0 slices of cos/sin on sync behind hid chunk 0
            nc.sync.dma_start(cos_sb[:, :CH], cos2[:, :CH])
            nc.sync.dma_start(sin_sb[:, :CH], sin2[:, :CH])
            nc.sync.dma_start(mask_sb[:], mask[:])

        def load_consts_bulk():
            # the scheduler hoists dependency-free DMAs to the front of
            # their queue, displacing the chunk-0/1 prefetch on the serial
            # DMA pipe.  Dummy reads gated on hid(1)'s first gulp add WAR
            # edges that hold these back until the cold stream has landed
            # (cos/sin cols 512-1408 are first needed by rope(1) at ~35us).
            junk = const.tile([128, 4], F16, tag="junk")
            sl = slice(CH, CH + 896)
            gate = hid8[1][:, :4]
            with nc.allow_low_precision(reason="dummy WAR gate"):
                nc.vector.tensor_add(junk[:, :1], wo8_sb[:, :1], gate[:, :1])
                nc.vector.tensor_add(junk[:, 1:2],
                                     cos_sb[:, sl.start:sl.start + 1],
                                     gate[:, 1:2])
                nc.vector.tensor_add(junk[:, 2:3],
                                     sin_sb[:, sl.start:sl.start + 1],
                                     gate[:, 2:3])
            nc.scalar.dma_start(cos_sb[:, sl], cos2[:, sl])
            nc.scalar.dma_start(sin_sb[:, sl], sin2[:, sl])
            nc.scalar.dma_start(wo8_sb[:], wo8[:])

        def load_consts_late():
            # sync queue behind step-2's first out-DMAs (which wait on their
            # staging evictions), so these never touch the cold-start pipe
            for u in range(1, 4):
                sl = slice(CH + u * 896, CH + (u + 1) * 896)
                nc.sync.dma_start(cos_sb[:, sl], cos2[:, sl])
                nc.sync.dma_start(sin_sb[:, sl], sin2[:, sl])


        # ---- persistent / ring activation tiles ----
        # KTR holds all 8 chunks; QTR/AT only live 1-2 steps -> rings of 2.
        KTR = [[persist.tile([128, CH], F16, tag=f"ktr{h}_{c}",
                             name=f"ktr{h}_{c}")
                for c in range(NCH)] for h in range(HPC)]
        QTR = [[None] * NCH for _ in range(HPC)]
        AT8 = [None] * NCH        # (hi, lo) fp8 [128, HPC*CH] per chunk
        VV = persist.tile([128, HPC * T], F16, tag="vv", name="vv")

        hid8 = [None] * NCH
        hid3 = [None] * NCH       # (hi_view, lo_view) [128, KT, CH]

        def alloc_hid(c):
            hid8[c] = hidp.tile([128, 2 * KT * CH], F8, tag="hid8",
                                name=f"hid8_{c}")
            v4 = hid8[c][:].rearrange("p (k two t) -> p two k t",
                                      two=2, k=KT)
            hid3[c] = (v4[:, 0], v4[:, 1])

        def load_hid(c):
            alloc_hid(c)
            # 4 gulps of 4 k-tiles on the Pool queue: SWDGE issues ~3x
            # cheaper than HWDGE for these multi-row-group descriptors, and
            # queue position gates prefetches behind earlier pool work
            for g in range(4):
                load_hid_gulp(c, g)

        def load_hid_gulp(c, g, k0=None, nk=None):
            if k0 is None:
                k0, nk = 4 * g, 4
            hid_v = hidT8[k0 * 256:(k0 + nk) * 256,
                          c * CH:(c + 1) * CH].rearrange(
                "(kt p) t -> p kt t", p=128)
            nc.gpsimd.dma_start(
                hid8[c][:, k0 * 2 * CH:(k0 + nk) * 2 * CH].rearrange(
                    "p (kt t) -> p kt t", kt=2 * nk), hid_v)

        # ---------------- projection passes ----------------
        def rope_evict(c, ft, psap, split_raw=False):
            """Evict a finished Q/K chain psum AP: RoPE via rotate-half.
            split_raw emits the ACT copy in halves -- used once in step 0 so
            five blocked ACT entries overflow the 4-deep wait queue and stop
            the const DMAs from jumping ahead of the chunk-0 prefetch."""
            h = ft % 2
            raw = ropep.tile([128, CH], F16, tag="raw", name=f"raw{c}_{ft}")
            if split_raw:
                nc.scalar.copy(raw[:, :256], psap[:, :256])
                nc.scalar.copy(raw[:, 256:], psap[:, 256:])
            else:
                nc.scalar.copy(raw[:], psap)
            rot = ropep.tile([128, CH], F16, tag="rot", name=f"rot{c}_{ft}")
            # sync/HWDGE queue: the pool SWDGE queue carries all prefetch
            # traffic, which would delay these small rotates by ~10us+
            nc.sync.dma_start(rot[0:64, :], raw[64:128, :])
            nc.sync.dma_start(rot[64:128, :], raw[0:64, :])
            ta = ropep.tile([128, CH], F16, tag="ta", name=f"ta{c}_{ft}")
            tb = ropep.tile([128, CH], F16, tag="tb", name=f"tb{c}_{ft}")
            csl = slice(c * CH, (c + 1) * CH)
            nc.vector.tensor_mul(ta[:], raw[:], cos_sb[:, csl])
            nc.vector.tensor_mul(tb[:], rot[:], sin_sb[:, csl])
            if ft < 2:
                dst = QTR[h][c] = qkp.tile([128, CH], F16, tag=f"qtr{h}",
                                           name=f"qtr{h}_{c}")
            else:
                dst = KTR[h][c]
            nc.vector.tensor_add(dst[:], ta[:], tb[:])
            return raw

        def qk_triplet(c, ft, psap, kp):
            """Compensated fp8 DoubleRow over k-tile pair (kp, kp+1).
            psap is a [128, CH] psum AP (tile slice)."""
            xh, xl = hid3[c]
            wsl = slice(ft * 128, (ft + 1) * 128)
            nc.tensor.matmul(psap, WH3[:, kp:kp + 2, wsl],
                             xh[:, kp:kp + 2, :],
                             start=(kp == 0), stop=False, perf_mode=DR)
            nc.tensor.matmul(psap, WL3[:, kp:kp + 2, wsl],
                             xh[:, kp:kp + 2, :],
                             start=False, stop=False, perf_mode=DR)
            nc.tensor.matmul(psap, WH3[:, kp:kp + 2, wsl],
                             xl[:, kp:kp + 2, :],
                             start=False, stop=(kp == KT - 2), perf_mode=DR)

        def qk_pass(c, pi, pump, ring2=False):
            """Two Q/K chains (head-pi's q and k) over all 16 k-tiles.
            ring2 puts the k chain in the (free) pw bank so it needn't wait
            for the q chain's rope eviction -- steps 1/7 only."""
            for i, ft in enumerate((pi, pi + 2)):          # q_h, k_h
                if ring2 and i == 1:
                    ps = psB.tile([128, CH], F32, tag="pw", bufs=2,
                                  name=f"psqk{c}_{ft}")
                else:
                    ps = psB.tile([128, CH], F32, tag="qv",
                                  name=f"psqk{c}_{ft}")
                for kp in range(0, KT, 2):
                    qk_triplet(c, ft, ps[:], kp)
                rope_evict(c, ft, ps[:])

        def v_unit(c, pi, kp, state):
            """One k-tile pair of the two V chains (t-tiles j=2*pi, 2*pi+1),
            each accumulating in its own PSUM bank (one group per zero
            region), compensated fp8 DoubleRow."""
            if 'a' not in state:
                state['a'] = psB.tile([128, 256], F32, tag="qv",
                                      name=f"psva{c}_{pi}")
                state['b'] = psB.tile([128, 256], F32, tag="pw", bufs=2,
                                      name=f"psvb{c}_{pi}")
            xh, xl = hid3[c]
            for jj, ps in ((0, state['a']), (1, state['b'])):
                j = 2 * pi + jj
                tsl = slice(j * 128, (j + 1) * 128)
                nc.tensor.matmul(ps[:], xh[:, kp:kp + 2, tsl],
                                 WVH3[:, kp:kp + 2, :],
                                 start=(kp == 0), stop=False, perf_mode=DR)
                nc.tensor.matmul(ps[:], xl[:, kp:kp + 2, tsl],
                                 WVH3[:, kp:kp + 2, :],
                                 start=False, stop=False, perf_mode=DR)
                nc.tensor.matmul(ps[:], xh[:, kp:kp + 2, tsl],
                                 WVL3[:, kp:kp + 2, :],
                                 start=False, stop=(kp == KT - 2),
                                 perf_mode=DR)
            if kp == KT - 2:
                # VV keeps the x32 wqkv pre-scale: PV's po comes out as
                # 32*attn, exactly the fp8 range needed for the wo split.
                for jj, ps in ((0, state['a']), (1, state['b'])):
                    kt_ = 4 * c + 2 * pi + jj
                    nc.vector.tensor_copy(VV[:, kt_ * 256:(kt_ + 1) * 256],
                                          ps[:])

        def v_units(c, pi):
            state = {}
            for kp in range(0, KT, 2):
                yield lambda kp=kp: v_unit(c, pi, kp, state)

        def v_pass(c, pi, pump):
            state = {}
            for kp in range(0, KT, 2):
                v_unit(c, pi, kp, state)

        # ---------------- attention ----------------
        def attn_head(a, h, pump, max_pumps=10 ** 9, wide=False, pump_from=0,
                      pump2=False):
            """Causal attention for chunk a, head h. S^T layout in 2-bank
            pair tiles so the exp of two k-tiles is ONE ACT instruction
            (halves the ACT per-instruction access overhead); diagonal
            pairs keep per-half exps to avoid exp'ing unwritten psum.
            Denominator on DVE+GPSIMD, PV accumulated in PSUM."""
            nkt = 4 * (a + 1)
            lag = LAG
            pumped = [0]

            def pump_(k):
                if pumped[0] < max_pumps:
                    pumped[0] += k
                    pump(k)
            po = psO.tile([128, CH], F32, tag="o", name=f"po{h}_{a}")
            acc = accp.tile([128, CH], F16, tag=f"acc{h}", name=f"acc{h}_{a}")
            pend = deque()

            def s_pair(kt0):
                ps = psS.tile([128, 2 * CH], F32, tag="s", bufs=2,
                              name=f"pss{h}_{a}_{kt0}")
                pt = ptp.tile([128, 2 * CH], F16, tag="pt", bufs=3,
                              name=f"pt{h}_{a}_{kt0}")
                halves = []
                for i, kt in enumerate((kt0, kt0 + 1)):
                    if kt >= nkt:
                        break
                    r = kt - 4 * a
                    qo = 128 * r if r > 0 else 0
                    off = i * CH
                    nc.tensor.matmul(
                        ps[:, off + qo:off + CH],
                        KTR[h][kt // 4][:, (kt % 4) * 128:(kt % 4 + 1) * 128],
                        QTR[h][a][:, qo:],
                        start=True, stop=True)
                    if r >= 0:
                        nc.vector.tensor_add(
                            ps[:, off + qo:off + qo + 128],
                            ps[:, off + qo:off + qo + 128], mask_sb[:])
                    halves.append((kt, qo, off))
                if len(halves) == 2 and all(kt < 4 * a for kt, _, _ in halves):
                    nc.scalar.activation(pt[:], ps[:],
                                         mybir.ActivationFunctionType.Exp,
                                         scale=SCALE)
                else:
                    for kt, qo, off in halves:
                        nc.scalar.activation(
                            pt[:, off + qo:off + CH], ps[:, off + qo:off + CH],
                            mybir.ActivationFunctionType.Exp, scale=SCALE)
                # denominator accumulation on DVE (off the PE)
                for kt, qo, off in halves:
                    if kt == 0:
                        nc.vector.tensor_copy(acc[:, qo:],
                                              pt[:, off + qo:off + CH])
                    else:
                        nc.vector.tensor_add(acc[:, qo:], acc[:, qo:],
                                             pt[:, off + qo:off + CH])
                return [(kt, qo, pt, off) for kt, qo, off in halves]

            def pv(pend_item):
                kt, qo, pt, off = pend_item
                nc.tensor.matmul(
                    po[:, qo:],
                    VV[:, kt * 256 + h * 128: kt * 256 + (h + 1) * 128],
                    pt[:, off + qo:off + CH],
                    start=(kt == 0), stop=(kt == nkt - 1))

            for kt0 in range(0, nkt, 2):
                pend.extend(s_pair(kt0))
                while len(pend) > lag:
                    pv(pend.popleft())
                if kt0 >= pump_from:
                    pump_(2 if pump2 else 1)
            while pend:
                pump_(1)
                pv(pend.popleft())

            def finish_norm(split=False):
                # normalize: at = po * (1 / sum_k exp), then split to fp8
                # hi (ACT) + lo (DVE) for the compensated DoubleRow w_o.
                # po carries the x32 V pre-scale, so `at` = 32*attn lands
                # in fp8's sweet spot.  Deferred by the caller so the reduce
                # never stalls the PE at a segment boundary.  split=True
                # pipelines the chain in column halves so the consumer
                # (tail w_o) can start on the first half early.
                dsum = accp.tile([128, CH], F32, tag=f"dsum{h}",
                                 name=f"ds{h}_{a}")
                binv = accp.tile([128, CH], F32, tag=f"binv{h}",
                                 name=f"bi{h}_{a}")
                if AT8[a] is None:
                    AT8[a] = (atp.tile([128, HPC * CH], F8, tag="at8h",
                                       name=f"at8h_{a}"),
                              atp.tile([128, HPC * CH], F8, tag="at8l",
                                       name=f"at8l_{a}"))
                at_hi, at_lo = AT8[a]
                th = accp.tile([128, CH], F16, tag=f"att{h}",
                               name=f"att{h}_{a}")
                for sl in ([slice(0, 128), slice(128, 256), slice(256, CH)]
                           if split else [slice(0, CH)]):
                    hsl = slice(h * CH + sl.start, h * CH + sl.stop)
                    nc.gpsimd.partition_all_reduce(
                        dsum[:, sl], acc[:, sl], channels=128,
                        reduce_op=bass_isa.ReduceOp.add)
                    with nc.allow_low_precision(reason="softmax 1/denom"):
                        nc.vector.reciprocal(binv[:, sl], dsum[:, sl])
                        nc.vector.tensor_mul(th[:, sl], po[:, sl],
                                             binv[:, sl])
                        nc.scalar.activation(
                            at_hi[:, hsl], th[:, sl],
                            mybir.ActivationFunctionType.Copy)
                        nc.vector.tensor_sub(at_lo[:, hsl], th[:, sl],
                                             at_hi[:, hsl])

            return finish_norm

        # ---------------- output projection ----------------
        OSCALE = 1.0 / (WSCALE * WSCALE)   # at and wo both carry x32

        def wo_units(c, dve_only=False, deep_ring=False, borrow_qv=False):
            """Yield filler closures: w_o projection of chunk c, one 512-col
            n-chunk (3 fp8 DoubleRow matmuls, head pair in the slot dim) at
            a time; eviction alternates DVE/ACT and descales by 1/1024.
            borrow_qv cycles the idle qv bank into the pw ring; deep_ring
            additionally borrows the S-ring halves (tail of the kernel)."""
            at_hi, at_lo = AT8[c]
            ah = at_hi[:].rearrange("p (h q) -> p h q", h=HPC)
            al = at_lo[:].rearrange("p (h q) -> p h q", h=HPC)
            uctr = [0]
            wide = [None]

            def get_pw(tt, n):
                i = uctr[0]
                uctr[0] += 1
                if deep_ring:
                    k = i % 4
                    if k in (0, 1):
                        if k == 0:
                            wide[0] = psS.tile([128, 2 * CH], F32, tag="s",
                                               bufs=2, name=f"pww{tt}_{n}")
                        return wide[0][:, k * CH:(k + 1) * CH]
                    if k == 2:
                        return psB.tile([128, CH], F32, tag="pw", bufs=2,
                                        name=f"pw{tt}_{n}")[:]
                    return psB.tile([128, CH], F32, tag="qv",
                                    name=f"pwq{tt}_{n}")[:]
                if borrow_qv and i % 3 == 2:
                    return psB.tile([128, CH], F32, tag="qv",
                                    name=f"pwq{tt}_{n}")[:]
                return psB.tile([128, CH], F32, tag="pw", bufs=2,
                                name=f"pw{tt}_{n}")[:]
            for j in range(4):
                tt = 4 * c + j
                stg = stgp.tile([128, HID], F16, tag="stg", name=f"stg{tt}")
                jsl = slice(j * 128, (j + 1) * 128)

                def unit(n, j=j, tt=tt, stg=stg, jsl=jsl):
                    pw = get_pw(tt, n)
                    nsl = slice(n * CH, (n + 1) * CH)
                    nc.tensor.matmul(pw, ah[:, :, jsl], WOH[:, :, nsl],
                                     start=True, stop=False, perf_mode=DR)
                    nc.tensor.matmul(pw, al[:, :, jsl], WOH[:, :, nsl],
                                     start=False, stop=False, perf_mode=DR)
                    nc.tensor.matmul(pw, ah[:, :, jsl], WOL[:, :, nsl],
                                     start=False, stop=True, perf_mode=DR)
                    if dve_only or n % 2 == 0:
                        nc.vector.tensor_scalar_mul(stg[:, nsl], pw, OSCALE)
                    else:
                        nc.scalar.activation(
                            stg[:, nsl], pw,
                            mybir.ActivationFunctionType.Copy, scale=OSCALE)
                    if n % 2 == 1:
                        nc.sync.dma_start(
                            out[tt * 128:(tt + 1) * 128,
                                (n - 1) * CH:(n + 1) * CH],
                            stg[:, (n - 1) * CH:(n + 1) * CH])

                for n in range(4):
                    yield lambda n=n, u=unit: u(n)

        # ---------------- main pipeline ----------------
        fillers = deque()

        def pump(k):
            for _ in range(k):
                if fillers:
                    fillers.popleft()()

        def pump_all():
            while fillers:
                fillers.popleft()()

        nop = lambda k: None

        # step 0: projection of chunk 0 as a wavefront -- all 8 chains consume
        # each (wqkv[kt], hid[kt]) DMA arrival together to hide cold-start
        # pacing.  qk chains borrow the idle S-ring PSUM banks.
        # cold-start: wqkv and hid0 gulps interleaved on the Pool queue so the
        # serial DMA pipe delivers (wqkv[kt], hid[kt]) pairs in consumption
        # order for the wavefront below
        # q/k weight columns for all 16 k-tiles land first (the qk wavefront
        # then outpaces no supply), V columns follow for the later V sweep
        alloc_hid(0)
        # gulp sizes balance SWDGE gen (994ns fixed each, serial on the
        # Pool engine) against first-arrival latency: small first gulp
        load_wqkv_gulp(0, nk=2)
        load_hid_gulp(0, 0)
        load_wqkv_gulp(2, nk=4)
        load_hid_gulp(0, 1)
        load_wqkv_gulp(6, nk=4)
        load_hid_gulp(0, 2)
        load_wqkv_gulp(10, nk=6)
        load_hid_gulp(0, 3)
        load_wqkv_v(0)
        load_wqkv_v(1)
        load_consts()
        qk0w = [psS.tile([128, 2 * CH], F32, tag="s", bufs=2,
                         name=f"ps0qk{w}") for w in range(2)]
        qk0 = [qk0w[ft // 2][:, (ft % 2) * CH:(ft % 2 + 1) * CH]
               for ft in range(4)]
        v0 = [psB.tile([128, 256], F32, tag="qv", name="ps0v0"),
              psB.tile([128, 256], F32, tag="pw", bufs=2, name="ps0v1"),
              psB.tile([128, 256], F32, tag="pw", bufs=2, name="ps0v2"),
              psO.tile([128, 256], F32, tag="o", name="ps0v3")]
        # prefetch chunk 1 now, WAR-gated on the chunk-0 qk chains: the
        # scheduler hoists dependency-free DMAs, and ungated these 4 gulps
        # would occupy the serial pipe ahead of the chunk-0 rope rotates
        # (rope -> attn(0) is the critical chain at the step-0/1 boundary)
        alloc_hid(1)
        junk2 = const.tile([128, 4], F16, tag="junk2")
        with nc.allow_low_precision(reason="dummy WAR gate"):
            for g in range(4):
                nc.vector.tensor_add(
                    junk2[:, g:g + 1], hid8[1][:, g * 4096:g * 4096 + 1],
                    qk0w[1][:, 1023:1024])
        for g in range(4):
            load_hid_gulp(1, g)
        # qk chains complete first so RoPE (raw->rot DMA->muls) starts as
        # early as possible; V sweeps the already-resident tiles second.
        for kp in range(0, KT, 2):
            for ft in range(4):
                qk_triplet(0, ft, qk0[ft], kp)
        raws0 = [rope_evict(0, ft, qk0[ft]) for ft in range(4)]
        xh0, xl0 = hid3[0]
        for kp in range(0, KT, 2):
            for j in range(4):
                tsl = slice(j * 128, (j + 1) * 128)
                nc.tensor.matmul(v0[j][:], xh0[:, kp:kp + 2, tsl],
                                 WVH3[:, kp:kp + 2, :],
                                 start=(kp == 0), stop=False, perf_mode=DR)
                nc.tensor.matmul(v0[j][:], xl0[:, kp:kp + 2, tsl],
                                 WVH3[:, kp:kp + 2, :],
                                 start=False, stop=False, perf_mode=DR)
                nc.tensor.matmul(v0[j][:], xh0[:, kp:kp + 2, tsl],
                                 WVL3[:, kp:kp + 2, :],
                                 start=False, stop=(kp == KT - 2),
                                 perf_mode=DR)
        for j in range(4):
            nc.vector.tensor_copy(VV[:, j * 256:(j + 1) * 256], v0[j][:])

        # step 1: attn(0) first -- it needs only chunk-0 rope, so its
        # exp-latency drains overlap the hid(1) prefetch still in the DMA
        # pipe; the chunk-1 projections follow once hid(1) has landed.
        # consts_bulk MUST be emitted before qk_pass(1,*): rope(1) reads
        # cos/sin columns it writes (emission order = dependency order).
        load_consts_bulk()
        fin0 = attn_head(0, 0, nop)
        fin0()
        fin1 = attn_head(0, 1, nop)
        fin1()
        qk_pass(1, 0, nop, ring2=True)
        load_hid(2)
        qk_pass(1, 1, nop, ring2=True)
        v_pass(1, 0, nop)
        v_pass(1, 1, nop)

        # steps 2..6: attn(c-1) + projection(c) + wo(c-2) as filler
        for c in range(2, NCH - 1):
            fillers.extend(wo_units(c - 2))
            fin0 = attn_head(c - 1, 0, pump)
            if c == 2:
                load_consts_late()

            qk_pass(c, 0, pump)
            fin0()
            load_hid(c + 1)
            fin1 = attn_head(c - 1, 1, pump)
            qk_pass(c, 1, pump)
            fin1()
            v_pass(c, 0, pump)
            v_pass(c, 1, pump)

        # step 7: attn(6) + projections of chunk 7, then attn(7,0)
        # immediately (it needs only chunk-7 q0/k0 rope, ready after
        # qk_pass(7,0)) with v(7)+wo(5) as filler -- this streams half of
        # the final chunk's exp while the PE still has projection work,
        # instead of leaving it for an exp-bound tail step.
        fillers.extend(wo_units(5))
        fin0 = attn_head(6, 0, pump)
        qk_pass(7, 0, pump)
        fin0()
        fin1 = attn_head(6, 1, pump)
        qk_pass(7, 1, pump)
        fin1()
        fillers.extend(v_units(7, 0))
        fillers.extend(v_units(7, 1))
        fin70 = attn_head(7, 0, pump)
        pump_all()
        fin70()

        # step 8: attn(7,1) + wo(6) + wo(7); wo(6) evictions DVE-only
        # (ACT is exp-heavy here), wo(7) alternates DVE/ACT for the drain.
        fillers.extend(wo_units(6, dve_only=True, borrow_qv=True))
        fin71 = attn_head(7, 1, pump, pump2=True)
        pump_all()
        fin71(split=True)
        fillers.extend(wo_units(7, deep_ring=True))
        pump_all()


_NC_CACHE = None


def _get_nc():
    global _NC_CACHE
    if _NC_CACHE is None:
        _NC_CACHE = _build_bass()
    return _NC_CACHE


def _f16(x):
    return np.ascontiguousarray(x).astype(np.float16)


def _split8_pack(x):
    """Error-compensated fp8 split, k-tile-interleaved: x [R, C] with
    R = nk*128 -> [2R, C] fp8 with row order (kt, {hi,lo}, p)."""
    x = np.ascontiguousarray(x, dtype=np.float32)
    hi = x.astype(NPF8)
    lo = (x - hi.astype(np.float32)).astype(NPF8)
    nk = x.shape[0] // 128
    packed = np.stack([hi.reshape(nk, 128, -1), lo.reshape(nk, 128, -1)],
                      axis=1)
    return np.ascontiguousarray(packed.reshape(2 * x.shape[0], x.shape[1]))


def prepare_inputs(hidden_states, positions, w_qkv, w_o):
    """Host-side sharding/preprocessing -> list of per-core input maps."""
    hidden_states = np.asarray(hidden_states, dtype=np.float32)
    positions = np.asarray(positions)
    w_qkv = np.asarray(w_qkv, dtype=np.float32)
    w_o = np.asarray(w_o, dtype=np.float32)

    hidT8 = _split8_pack(hidden_states.T)

    pos = positions.astype(np.float32)
    half = HD // 2
    inv_freq = 1.0 / (10000.0 ** (np.arange(half, dtype=np.float32) / half))
    freqs = np.outer(pos, inv_freq)          # [T, 64]
    cos = np.cos(freqs).T / WSCALE           # [64, T]; undo the wqkv fp8
    sin = np.sin(freqs).T / WSCALE           # pre-scale during RoPE
    cos2 = _f16(np.concatenate([cos, cos], axis=0))
    sin2 = _f16(np.concatenate([-sin, sin], axis=0))

    # single [128, 128] causal block: 0 where k <= q, else -3e4
    k_idx = np.arange(128)[:, None]
    q_idx = np.arange(128)[None, :]
    mask_np = np.where(k_idx <= q_idx, 0.0, MASK_NEG).astype(np.float32)

    in_maps = []
    for core in range(NCORES):
        heads = [HPC * core + i for i in range(HPC)]
        wq = [w_qkv[:, h * HD:(h + 1) * HD] for h in heads]
        wk = [w_qkv[:, FEAT * NCORES + h * HD:FEAT * NCORES + (h + 1) * HD]
              for h in heads]
        wv = [w_qkv[:, 2 * FEAT * NCORES + h * HD:2 * FEAT * NCORES + (h + 1) * HD]
              for h in heads]
        wqkv_core = np.concatenate(wq + wk + wv, axis=1) * WSCALE
        # v columns in partition-major SBUF-image layout [128, (k two f)]
        v_cols = np.ascontiguousarray(wqkv_core[:, 512:], dtype=np.float32)
        v_hi = v_cols.astype(NPF8)
        v_lo = (v_cols - v_hi.astype(np.float32)).astype(NPF8)
        wqkv8v = np.ascontiguousarray(
            np.stack([v_hi.reshape(KT, 128, 256), v_lo.reshape(KT, 128, 256)],
                     axis=1)                      # [KT, 2, 128, 256]
            .transpose(2, 0, 1, 3).reshape(128, 2 * KT * 256))
        # wo8: [128, (two, head, f)] fp8 hi/lo of 32*w_o rows per head
        wo_core = np.stack([w_o[h * HD:(h + 1) * HD, :] for h in heads],
                           axis=0) * WSCALE          # [HPC, 128, HID]
        wo_hi = wo_core.astype(NPF8)
        wo_lo = (wo_core - wo_hi.astype(np.float32)).astype(NPF8)
        wo8 = np.ascontiguousarray(
            np.stack([wo_hi, wo_lo], axis=0)          # [two, HPC, 128, HID]
            .transpose(2, 0, 1, 3).reshape(128, 4 * HID))
        in_maps.append({
            "hidT8": hidT8,
            "wqkv8qk": _split8_pack(wqkv_core[:, :512]),
            "wqkv8v": wqkv8v,
            "wo8": wo8,
            "cos2": cos2,
            "sin2": sin2,
            "mask": mask_np,
        })
    return in_maps


def kernel(hidden_states, positions, w_qkv, w_o):
    in_maps = prepare_inputs(hidden_states, positions, w_qkv, w_o)
    nc = _get_nc()
    try:
        res = run_bass_kernel_spmd(nc, in_maps, core_ids=list(range(NCORES)))
    except Exception:
        # transient device wedge from a prior crashed process: retry once
        res = run_bass_kernel_spmd(nc, in_maps, core_ids=list(range(NCORES)))
    acc = res.results[0]["out"].astype(np.float32)
    for i in range(1, NCORES):
        acc += res.results[i]["out"].astype(np.float32)
    return acc

